# revision 15
# baseline (speedup 1.0000x reference)
"""Trainium2 Bass kernel for the AnalyticalBoundedLineAttractor problem.

Reference semantics (per step, per sample):
    z = x @ W.T + b;  m = (z > 0);  A = diag(m) @ W - I;  c = m * b
    x_next = expm(A*dt) @ x + (expm(A*dt) - I) @ pinv(A) @ c

Scheme: K=2 Taylor of the augmented matrix exponential (lam = exp(-dt)):
    p0  = dt*(W x + b);  v1 = lam*relu(p0) = relu(lam*p0)
    v2  = (v1 > 0) * ((dt/2) W v1 + lam*(dt^2/2) b)
    x'  = lam*x + v1 + v2

This is a LATENCY-bound problem: all 8 cores run the same serial
99-step recurrence, so wall time == per-step critical-path length.
The baseline chain was relu -> MM(W v1) -> mask -> MM(z') = 4 engine
ops (~912 ns/step).  Two one-step lags cut it to 2 ops (~460 ns):
  * the correction matmul B_t = (dt/2) W v1_{t-1} + c2 uses the
    PREVIOUS v1, so it runs off the chain, and
  * v2 enters the state one step late through the auxiliary state
    Xg_{t+1} = lam*x_{t+1} + v2_t  (x_{t+1} = Xg_t + v1_t).
Both lags are O(dt^3)-per-step perturbations, the same order as the
K=2 truncation (numpy check vs the expm/pinv reference: 1.4e-3 rel
err with fp16 rounding; the gate is 2e-2).

Per-step engine schedule (chain = ACT -> MM -> ACT, everything else
hangs off it with >= one period of slack):
    ACT   : V1_t = relu(lam * A_t)            [CHAIN]
    PE    : A_{t+1} = dtW@Xg_t (start) + dtW@V1_t (stop) + dt*b
            (only the V1 part is chain);  B_{t+1} = (dt/2)W@V1_t + c2
    Pool  : s_t = Xg_t + V1_t   (= x_{t+1}: the recorded trajectory)
    DVE   : v2_t = (V1_t > 0) * B_t;   Xg_{t+1} = lam*s_t + v2_t
The trajectory history s is streamed out by DMA during the loop.
Weights: one [65, 2*64] fp16 stack, bias row 64 (dt*b | lam*dt^2/2*b);
V1/Xg carry an augmented row (1/0) so the bias enters exactly once.
Per-core 32 samples, D=64 on partitions, fp16 state, fp32 PSUM.
"""

import math
import sys

import numpy as np

try:
    from concourse.bass_utils import run_bass_kernel_spmd
except ImportError:
    sys.path.insert(0, "/opt/trn_rl_repo")
    from concourse.bass_utils import run_bass_kernel_spmd

import concourse.bacc as bacc
import concourse.mybir as mybir
import concourse.tile as tile

DT = 0.05
T_STEPS = 100
DIM = 64
BATCH = 256
N_CORES = 8
BL = BATCH // N_CORES  # 32 samples per core
NT = T_STEPS - 1  # 99 loop steps
LAM = math.exp(-DT)
F32 = mybir.dt.float32
F16 = mybir.dt.float16

_CACHE = {}


def _build_nc():
    nc = bacc.Bacc(None, target_bir_lowering=False)
    x0_ext = nc.declare_dram_parameter("x0h", [DIM, BL], F16, isOutput=False)
    wts_ext = nc.declare_dram_parameter("wth", [DIM + 1, 2 * DIM], F16, isOutput=False)
    ones_ext = nc.declare_dram_parameter("oneh", [1, NT * BL], F16, isOutput=False)
    s_ext = nc.declare_dram_parameter("sh", [DIM, NT * BL], F16, isOutput=True)

    OP = mybir.AluOpType
    ACTF = mybir.ActivationFunctionType

    with tile.TileContext(nc) as tc:
        with (
            tc.tile_pool(name="sb", bufs=1) as sb,
            tc.tile_pool(name="ps", bufs=2, space="PSUM") as ps,
        ):
            wts = sb.tile([DIM + 1, 2 * DIM], F16)
            x0aug = sb.tile([DIM + 1, BL], F16)
            sH = sb.tile([DIM, NT * BL], F16)  # x_{t+1} history
            # append-only histories (fresh slot per step -> every consumer
            # sees a single-producer tile: one wait condition, no WAR sems
            # in front of the chain-critical relu).  V1h's bias row (all
            # ones) comes in by DMA -- a full-row memset costs ~2.7us on
            # DVE and would gate the first matmuls.
            V1h = sb.tile([DIM + 1, NT * BL], F16)  # row DIM = 1 (bias row)
            Xgh = sb.tile([DIM, (NT + 1) * BL], F16)  # no bias row needed
            v2 = [sb.tile([DIM, BL], F16, name=f"v2_{k}") for k in range(2)]

            nc.sync.dma_start(wts[:], wts_ext[:])
            nc.scalar.dma_start(x0aug[0:DIM, :], x0_ext[:])
            nc.sync.dma_start(V1h[DIM : DIM + 1, :], ones_ext[:])
            nc.vector.memset(x0aug[DIM : DIM + 1, :], 1.0)
            nc.vector.memset(v2[0][:], 0.0)
            nc.vector.memset(v2[1][:], 0.0)

            wZ = wts[:, 0:DIM]  # dt*W.T | row64 = dt*b
            wH = wts[:, DIM : 2 * DIM]  # (dt/2)*W.T | row64 = lam*dt^2/2*b

            # A_0 = dt(W x0 + b);  Xg_0 = lam*x0
            A_cur = ps.tile([DIM, BL], F32, name="A")
            nc.tensor.matmul(A_cur[:], wZ, x0aug[:], start=True, stop=True)
            nc.scalar.activation(
                Xgh[:, 0:BL], x0aug[0:DIM, :], ACTF.Copy, scale=LAM
            )

            for t in range(NT):
                sV = V1h[:, t * BL : (t + 1) * BL]
                sXg = Xgh[:, t * BL : (t + 1) * BL]
                s_slot = sH[:, t * BL : (t + 1) * BL]

                # [CHAIN] V1_t = relu(lam * A_t)
                nc.scalar.activation(sV[0:DIM, :], A_cur[:], ACTF.Relu, scale=LAM)

                # A_{t+1} = dtW@Xg_t + dtW@V1_t + dt*b.  The Xg part issues
                # early (its operand lands mid-relu) and drains; the V1 part
                # issues at the relu sem and is the only chain matmul.
                A_nxt = ps.tile([DIM, BL], F32, name="A")
                nc.tensor.matmul(A_nxt[:], wZ[0:DIM, :], sXg, start=True, stop=False)
                nc.tensor.matmul(A_nxt[:], wZ, sV, start=False, stop=True)

                # B_t = (dt/2)W@v1_{t-1} + c2: the operand is the PREVIOUS
                # step's v1 (the lag), so this matmul is ready at step start
                # and never blocks the chain A-matmuls on the in-order PE.
                if t > 0:
                    sVp = V1h[:, (t - 1) * BL : t * BL]
                    B_cur = ps.tile([DIM, BL], F32, name="B")
                    nc.tensor.matmul(B_cur[:], wH, sVp, start=True, stop=True)

                # s_t = Xg_t + V1_t  (= x_{t+1}, the recorded trajectory).
                # On DVE, right before the Xg update: same engine, in-order,
                # so the s -> Xg handoff needs no semaphore round-trip.
                nc.vector.tensor_tensor(s_slot, sXg, sV[0:DIM, :], op=OP.add)

                # Xg_{t+1} = lam*s_t + v2_{t-1}: v2 is double-buffered, so
                # this does NOT wait on this step's v2 -- one hop after sTT.
                if t < NT - 1:
                    sXg1 = Xgh[:, (t + 1) * BL : (t + 2) * BL]
                    nc.vector.scalar_tensor_tensor(
                        sXg1, s_slot, LAM, v2[(t - 1) % 2][:],
                        op0=OP.mult, op1=OP.add,
                    )

                # v2_t = (V1_t > 0) * B_t  (read by the Xg update next step)
                if t > 0:
                    nc.vector.scalar_tensor_tensor(
                        v2[t % 2][:], sV[0:DIM, :], 0.0, B_cur[:],
                        op0=OP.is_gt, op1=OP.mult,
                    )

                A_cur = A_nxt

                # stream finished trajectory chunks; small tail chunk
                bounds = {17: 0, 37: 18, 57: 38, 77: 58, 94: 78, 98: 95}
                if t in bounds:
                    lo, hi = bounds[t] * BL, (t + 1) * BL
                    nc.sync.dma_start(s_ext[:, lo:hi], sH[:, lo:hi])

    nc.compile()
    return nc


def _host_weights(W, b):
    """Stationary weight stack (DIM+1, 2*DIM) fp16; fp64 math then cast."""
    W64 = W.astype(np.float64)
    b64 = b.astype(np.float64)
    wts = np.zeros((DIM + 1, 2 * DIM), np.float64)
    wts[0:DIM, 0:DIM] = DT * W64.T
    wts[DIM, 0:DIM] = DT * b64
    wts[0:DIM, DIM : 2 * DIM] = (DT / 2) * W64.T
    wts[DIM, DIM : 2 * DIM] = LAM * (DT**2 / 2) * b64
    return np.ascontiguousarray(wts.astype(np.float16))


def _run_device(x0, W, b, **spmd_kwargs):
    if "nc" not in _CACHE:
        _CACHE["nc"] = _build_nc()
    nc = _CACHE["nc"]

    wts = _host_weights(W, b)
    ones = np.ones((1, NT * BL), np.float16)
    in_maps = []
    for i in range(N_CORES):
        shard = np.ascontiguousarray(
            x0[i * BL : (i + 1) * BL].T.astype(np.float16)
        )
        in_maps.append({"x0h": shard, "wth": wts, "oneh": ones})

    return run_bass_kernel_spmd(
        nc, in_maps, core_ids=list(range(N_CORES)), **spmd_kwargs
    )


def kernel(initial_position, W, b):
    x0 = np.asarray(initial_position, np.float32)
    W = np.asarray(W, np.float32)
    b = np.asarray(b, np.float32)

    res = _run_device(x0, W, b)

    out = np.empty((BATCH, T_STEPS, DIM), np.float32)
    for i in range(N_CORES):
        sh = res.results[i]["sh"].astype(np.float32)  # (DIM, NT*BL)
        out[i * BL : (i + 1) * BL, 0] = x0[i * BL : (i + 1) * BL]
        out[i * BL : (i + 1) * BL, 1:] = sh.reshape(DIM, NT, BL).transpose(2, 1, 0)
    return out


# revision 16
# speedup vs baseline: 1.3139x; 1.3139x over previous
"""Trainium2 Bass kernel for the AnalyticalBoundedLineAttractor problem.

Reference semantics (per step, per sample):
    z = x @ W.T + b;  m = (z > 0);  A = diag(m) @ W - I;  c = m * b
    x_next = expm(A*dt) @ x + (expm(A*dt) - I) @ pinv(A) @ c

Scheme: K=2 Taylor of the augmented matrix exponential (lam = exp(-dt)):
    p0  = dt*(W x + b);  v1 = lam*relu(p0) = relu(lam*p0)
    v2  = (v1 > 0) * ((dt/2) W v1 + lam*(dt^2/2) b)
    x'  = lam*x + v1 + v2

This is a LATENCY-bound problem: all 8 cores run the same serial
99-step recurrence, so wall time == per-step critical-path length.
The baseline chain was relu -> MM(W v1) -> mask -> MM(z') = 4 engine
ops (~912 ns/step).  Two one-step lags cut it to 2 ops (~460 ns):
  * the correction matmul B_t = (dt/2) W v1_{t-1} + c2 uses the
    PREVIOUS v1, so it runs off the chain, and
  * v2 enters the state one step late through the auxiliary state
    Xg_{t+1} = lam*x_{t+1} + v2_t  (x_{t+1} = Xg_t + v1_t).
Both lags are O(dt^3)-per-step perturbations, the same order as the
K=2 truncation (numpy check vs the expm/pinv reference: 1.4e-3 rel
err with fp16 rounding; the gate is 2e-2).

Per-step engine schedule (chain = ACT -> MM -> ACT, everything else
hangs off it with >= one period of slack):
    ACT   : V1_t = relu(lam * A_t)            [CHAIN]
    PE    : A_{t+1} = dtW@Xg_t (start) + dtW@V1_t (stop) + dt*b
            (only the V1 part is chain);  B_{t+1} = (dt/2)W@V1_t + c2
    Pool  : s_t = Xg_t + V1_t   (= x_{t+1}: the recorded trajectory)
    DVE   : v2_t = (V1_t > 0) * B_t;   Xg_{t+1} = lam*s_t + v2_t
The trajectory history s is streamed out by DMA during the loop.
Weights: one [65, 2*64] fp16 stack, bias row 64 (dt*b | lam*dt^2/2*b);
V1/Xg carry an augmented row (1/0) so the bias enters exactly once.
Per-core 32 samples, D=64 on partitions, fp16 state, fp32 PSUM.
"""

import math
import sys

import numpy as np

try:
    from concourse.bass_utils import run_bass_kernel_spmd
except ImportError:
    sys.path.insert(0, "/opt/trn_rl_repo")
    from concourse.bass_utils import run_bass_kernel_spmd

import concourse.bacc as bacc
import concourse.mybir as mybir
import concourse.tile as tile

DT = 0.05
T_STEPS = 100
DIM = 64
BATCH = 256
N_CORES = 8
BL = BATCH // N_CORES  # 32 samples per core
NT = T_STEPS - 1  # 99 loop steps
LAM = math.exp(-DT)
F32 = mybir.dt.float32
F16 = mybir.dt.float16

_CACHE = {}


def _build_nc():
    nc = bacc.Bacc(None, target_bir_lowering=False)
    x0_ext = nc.declare_dram_parameter("x0h", [DIM, BL], F16, isOutput=False)
    wts_ext = nc.declare_dram_parameter("wth", [DIM + 1, 2 * DIM], F16, isOutput=False)
    ones_ext = nc.declare_dram_parameter("oneh", [1, NT * BL], F16, isOutput=False)
    s_ext = nc.declare_dram_parameter("sh", [DIM, NT * BL], F16, isOutput=True)

    OP = mybir.AluOpType
    ACTF = mybir.ActivationFunctionType

    with tile.TileContext(nc) as tc:
        with (
            tc.tile_pool(name="sb", bufs=1) as sb,
            tc.tile_pool(name="ps", bufs=2, space="PSUM") as ps,
        ):
            wts = sb.tile([DIM + 1, 2 * DIM], F16)
            x0aug = sb.tile([DIM + 1, BL], F16)
            sH = sb.tile([DIM, NT * BL], F16)  # x_{t+1} history
            # append-only histories (fresh slot per step -> every consumer
            # sees a single-producer tile: one wait condition, no WAR sems
            # in front of the chain-critical relu).  V1h's bias row (all
            # ones) comes in by DMA -- a full-row memset costs ~2.7us on
            # DVE and would gate the first matmuls.
            V1h = sb.tile([DIM + 1, NT * BL], F16)  # row DIM = 1 (bias row)
            Xgh = sb.tile([DIM, (NT + 1) * BL], F16)  # no bias row needed
            v2 = [sb.tile([DIM, BL], F16, name=f"v2_{k}") for k in range(2)]

            nc.sync.dma_start(wts[:], wts_ext[:])
            nc.scalar.dma_start(x0aug[0:DIM, :], x0_ext[:])
            nc.sync.dma_start(V1h[DIM : DIM + 1, :], ones_ext[:])
            nc.vector.memset(x0aug[DIM : DIM + 1, :], 1.0)
            nc.vector.memset(v2[0][:], 0.0)
            nc.vector.memset(v2[1][:], 0.0)

            wZ = wts[:, 0:DIM]  # dt*W.T | row64 = dt*b
            wH = wts[:, DIM : 2 * DIM]  # (dt/2)*W.T | row64 = lam*dt^2/2*b

            # A_0 = dt(W x0 + b);  Xg_0 = lam*x0
            A_cur = ps.tile([DIM, BL], F32, name="A")
            nc.tensor.matmul(A_cur[:], wZ, x0aug[:], start=True, stop=True)
            nc.scalar.activation(
                Xgh[:, 0:BL], x0aug[0:DIM, :], ACTF.Copy, scale=LAM
            )

            for t in range(NT):
                sV = V1h[:, t * BL : (t + 1) * BL]
                sXg = Xgh[:, t * BL : (t + 1) * BL]
                s_slot = sH[:, t * BL : (t + 1) * BL]

                # [CHAIN] V1_t = relu(lam * A_t)
                nc.scalar.activation(sV[0:DIM, :], A_cur[:], ACTF.Relu, scale=LAM)

                # A_{t+1} = dtW@Xg_t + dtW@V1_t + dt*b.  The Xg part issues
                # early (its operand lands mid-relu) and drains; the V1 part
                # issues at the relu sem and is the only chain matmul.
                A_nxt = ps.tile([DIM, BL], F32, name="A")
                nc.tensor.matmul(A_nxt[:], wZ[0:DIM, :], sXg, start=True, stop=False)
                nc.tensor.matmul(A_nxt[:], wZ, sV, start=False, stop=True)

                # B_t = (dt/2)W@v1_{t-1} + c2: the operand is the PREVIOUS
                # step's v1 (the lag), so this matmul is ready at step start
                # and never blocks the chain A-matmuls on the in-order PE.
                if t > 0:
                    sVp = V1h[:, (t - 1) * BL : t * BL]
                    B_cur = ps.tile([DIM, BL], F32, name="B")
                    nc.tensor.matmul(B_cur[:], wH, sVp, start=True, stop=True)

                # s_t = Xg_t + V1_t  (= x_{t+1}, the recorded trajectory;
                # consumed only by the Xg update and the output DMA)
                nc.gpsimd.tensor_tensor(s_slot, sXg, sV[0:DIM, :], op=OP.add)

                # Xg_{t+1} = lam*s_t + v2_{t-1}: v2 is double-buffered, so
                # this does NOT wait on this step's v2 -- one hop after sTT.
                if t < NT - 1:
                    sXg1 = Xgh[:, (t + 1) * BL : (t + 2) * BL]
                    nc.vector.scalar_tensor_tensor(
                        sXg1, s_slot, LAM, v2[(t - 1) % 2][:],
                        op0=OP.mult, op1=OP.add,
                    )

                # v2_t = (V1_t > 0) * B_t  (read by the Xg update next step)
                if t > 0:
                    nc.vector.scalar_tensor_tensor(
                        v2[t % 2][:], sV[0:DIM, :], 0.0, B_cur[:],
                        op0=OP.is_gt, op1=OP.mult,
                    )

                A_cur = A_nxt

                # stream finished trajectory chunks; small tail chunk
                bounds = {17: 0, 37: 18, 57: 38, 77: 58, 94: 78, 98: 95}
                if t in bounds:
                    lo, hi = bounds[t] * BL, (t + 1) * BL
                    nc.sync.dma_start(s_ext[:, lo:hi], sH[:, lo:hi])

    nc.compile()
    return nc


def _host_weights(W, b):
    """Stationary weight stack (DIM+1, 2*DIM) fp16; fp64 math then cast."""
    W64 = W.astype(np.float64)
    b64 = b.astype(np.float64)
    wts = np.zeros((DIM + 1, 2 * DIM), np.float64)
    wts[0:DIM, 0:DIM] = DT * W64.T
    wts[DIM, 0:DIM] = DT * b64
    wts[0:DIM, DIM : 2 * DIM] = (DT / 2) * W64.T
    wts[DIM, DIM : 2 * DIM] = LAM * (DT**2 / 2) * b64
    return np.ascontiguousarray(wts.astype(np.float16))


def _run_device(x0, W, b, **spmd_kwargs):
    if "nc" not in _CACHE:
        _CACHE["nc"] = _build_nc()
    nc = _CACHE["nc"]

    wts = _host_weights(W, b)
    ones = np.ones((1, NT * BL), np.float16)
    in_maps = []
    for i in range(N_CORES):
        shard = np.ascontiguousarray(
            x0[i * BL : (i + 1) * BL].T.astype(np.float16)
        )
        in_maps.append({"x0h": shard, "wth": wts, "oneh": ones})

    return run_bass_kernel_spmd(
        nc, in_maps, core_ids=list(range(N_CORES)), **spmd_kwargs
    )


def kernel(initial_position, W, b):
    x0 = np.asarray(initial_position, np.float32)
    W = np.asarray(W, np.float32)
    b = np.asarray(b, np.float32)

    res = _run_device(x0, W, b)

    out = np.empty((BATCH, T_STEPS, DIM), np.float32)
    for i in range(N_CORES):
        sh = res.results[i]["sh"].astype(np.float32)  # (DIM, NT*BL)
        out[i * BL : (i + 1) * BL, 0] = x0[i * BL : (i + 1) * BL]
        out[i * BL : (i + 1) * BL, 1:] = sh.reshape(DIM, NT, BL).transpose(2, 1, 0)
    return out


# revision 18
# speedup vs baseline: 1.3164x; 1.0019x over previous
"""Trainium2 Bass kernel for the AnalyticalBoundedLineAttractor problem.

Reference semantics (per step, per sample):
    z = x @ W.T + b;  m = (z > 0);  A = diag(m) @ W - I;  c = m * b
    x_next = expm(A*dt) @ x + (expm(A*dt) - I) @ pinv(A) @ c

Scheme: K=2 Taylor of the augmented matrix exponential (lam = exp(-dt)):
    p0  = dt*(W x + b);  v1 = lam*relu(p0) = relu(lam*p0)
    v2  = (v1 > 0) * ((dt/2) W v1 + lam*(dt^2/2) b)
    x'  = lam*x + v1 + v2

This is a LATENCY-bound problem: all 8 cores run the same serial
99-step recurrence, so wall time == per-step critical-path length.
The baseline chain was relu -> MM(W v1) -> mask -> MM(z') = 4 engine
ops (~912 ns/step).  Two one-step lags cut it to 2 ops (~460 ns):
  * the correction matmul B_t = (dt/2) W v1_{t-1} + c2 uses the
    PREVIOUS v1, so it runs off the chain, and
  * v2 enters the state one step late through the auxiliary state
    Xg_{t+1} = lam*x_{t+1} + v2_t  (x_{t+1} = Xg_t + v1_t).
Both lags are O(dt^3)-per-step perturbations, the same order as the
K=2 truncation (numpy check vs the expm/pinv reference: 1.4e-3 rel
err with fp16 rounding; the gate is 2e-2).

Per-step engine schedule (chain = ACT -> MM -> ACT, everything else
hangs off it with >= one period of slack):
    ACT   : V1_t = relu(lam * A_t)            [CHAIN]
    PE    : A_{t+1} = dtW@Xg_t (start) + dtW@V1_t (stop) + dt*b
            (only the V1 part is chain);  B_{t+1} = (dt/2)W@V1_t + c2
    Pool  : s_t = Xg_t + V1_t   (= x_{t+1}: the recorded trajectory)
    DVE   : v2_t = (V1_t > 0) * B_t;   Xg_{t+1} = lam*s_t + v2_t
The trajectory history s is streamed out by DMA during the loop.
Weights: one [65, 2*64] fp16 stack, bias row 64 (dt*b | lam*dt^2/2*b);
V1/Xg carry an augmented row (1/0) so the bias enters exactly once.
Per-core 32 samples, D=64 on partitions, fp16 state, fp32 PSUM.
"""

import math
import sys

import numpy as np

try:
    from concourse.bass_utils import run_bass_kernel_spmd
except ImportError:
    sys.path.insert(0, "/opt/trn_rl_repo")
    from concourse.bass_utils import run_bass_kernel_spmd

import concourse.bacc as bacc
import concourse.mybir as mybir
import concourse.tile as tile

DT = 0.05
T_STEPS = 100
DIM = 64
BATCH = 256
N_CORES = 8
BL = BATCH // N_CORES  # 32 samples per core
NT = T_STEPS - 1  # 99 loop steps
LAM = math.exp(-DT)
F32 = mybir.dt.float32
F16 = mybir.dt.float16

_CACHE = {}


def _build_nc():
    nc = bacc.Bacc(None, target_bir_lowering=False)
    x0_ext = nc.declare_dram_parameter("x0h", [DIM, BL], F16, isOutput=False)
    wts_ext = nc.declare_dram_parameter("wth", [DIM + 1, 2 * DIM], F16, isOutput=False)
    ones_ext = nc.declare_dram_parameter("oneh", [1, NT * BL], F16, isOutput=False)
    s_ext = nc.declare_dram_parameter("sh", [DIM, NT * BL], F16, isOutput=True)

    OP = mybir.AluOpType
    ACTF = mybir.ActivationFunctionType

    with tile.TileContext(nc) as tc:
        with (
            tc.tile_pool(name="sb", bufs=1) as sb,
            tc.tile_pool(name="ps", bufs=2, space="PSUM") as ps,
        ):
            wts = sb.tile([DIM + 1, 2 * DIM], F16)
            x0aug = sb.tile([DIM + 1, BL], F16)
            sH = sb.tile([DIM, NT * BL], F16)  # x_{t+1} history
            # append-only histories (fresh slot per step -> every consumer
            # sees a single-producer tile: one wait condition, no WAR sems
            # in front of the chain-critical relu).  V1h's bias row (all
            # ones) comes in by DMA -- a full-row memset costs ~2.7us on
            # DVE and would gate the first matmuls.
            V1h = sb.tile([DIM + 1, NT * BL], F16)  # row DIM = 1 (bias row)
            Xgh = sb.tile([DIM, (NT + 1) * BL], F16)  # no bias row needed
            v2 = [sb.tile([DIM, BL], F16, name=f"v2_{k}") for k in range(2)]

            nc.sync.dma_start(wts[:], wts_ext[:])
            nc.scalar.dma_start(x0aug[0:DIM, :], x0_ext[:])
            nc.sync.dma_start(V1h[DIM : DIM + 1, :], ones_ext[:])
            nc.vector.memset(x0aug[DIM : DIM + 1, :], 1.0)
            nc.vector.memset(v2[0][:], 0.0)
            nc.vector.memset(v2[1][:], 0.0)

            wZ = wts[:, 0:DIM]  # dt*W.T | row64 = dt*b
            wH = wts[:, DIM : 2 * DIM]  # (dt/2)*W.T | row64 = lam*dt^2/2*b

            # A_0 = dt(W x0 + b);  Xg_0 = lam*x0
            A_cur = ps.tile([DIM, BL], F32, name="A")
            nc.tensor.matmul(A_cur[:], wZ, x0aug[:], start=True, stop=True)
            nc.scalar.activation(
                Xgh[:, 0:BL], x0aug[0:DIM, :], ACTF.Copy, scale=LAM
            )

            for t in range(NT):
                sV = V1h[:, t * BL : (t + 1) * BL]
                sXg = Xgh[:, t * BL : (t + 1) * BL]
                s_slot = sH[:, t * BL : (t + 1) * BL]

                # [CHAIN] V1_t = relu(A_t)  (lam folded into wZ on the host)
                nc.scalar.activation(sV[0:DIM, :], A_cur[:], ACTF.Relu)

                # A_{t+1} = dtW@Xg_t + dtW@V1_t + dt*b.  The Xg part issues
                # early (its operand lands mid-relu) and drains; the V1 part
                # issues at the relu sem and is the only chain matmul.
                A_nxt = ps.tile([DIM, BL], F32, name="A")
                nc.tensor.matmul(A_nxt[:], wZ[0:DIM, :], sXg, start=True, stop=False)
                nc.tensor.matmul(A_nxt[:], wZ, sV, start=False, stop=True)

                # B_t = (dt/2)W@v1_{t-1} + c2: the operand is the PREVIOUS
                # step's v1 (the lag), so this matmul is ready at step start
                # and never blocks the chain A-matmuls on the in-order PE.
                if t > 0:
                    sVp = V1h[:, (t - 1) * BL : t * BL]
                    B_cur = ps.tile([DIM, BL], F32, name="B")
                    nc.tensor.matmul(B_cur[:], wH, sVp, start=True, stop=True)

                # s_t = Xg_t + V1_t  (= x_{t+1}, the recorded trajectory;
                # consumed only by the Xg update and the output DMA)
                nc.gpsimd.tensor_tensor(s_slot, sXg, sV[0:DIM, :], op=OP.add)

                # Xg_{t+1} = lam*s_t + v2_{t-1}: v2 is double-buffered, so
                # this does NOT wait on this step's v2 -- one hop after sTT.
                if t < NT - 1:
                    sXg1 = Xgh[:, (t + 1) * BL : (t + 2) * BL]
                    nc.vector.scalar_tensor_tensor(
                        sXg1, s_slot, LAM, v2[(t - 1) % 2][:],
                        op0=OP.mult, op1=OP.add,
                    )

                # v2_t = (V1_t > 0) * B_t  (read by the Xg update next step)
                if t > 0:
                    nc.vector.scalar_tensor_tensor(
                        v2[t % 2][:], sV[0:DIM, :], 0.0, B_cur[:],
                        op0=OP.is_gt, op1=OP.mult,
                    )

                A_cur = A_nxt

                # stream finished trajectory chunks; small tail chunk
                bounds = {17: 0, 37: 18, 57: 38, 77: 58, 94: 78, 98: 95}
                if t in bounds:
                    lo, hi = bounds[t] * BL, (t + 1) * BL
                    nc.sync.dma_start(s_ext[:, lo:hi], sH[:, lo:hi])

    nc.compile()
    return nc


def _host_weights(W, b):
    """Stationary weight stack (DIM+1, 2*DIM) fp16; fp64 math then cast."""
    W64 = W.astype(np.float64)
    b64 = b.astype(np.float64)
    wts = np.zeros((DIM + 1, 2 * DIM), np.float64)
    # lam is folded into the z block so the relu runs with scale=1:
    # A = lam*dt*(W x + b)  =>  relu(A) = lam*relu(dt(Wx+b)) = v1
    wts[0:DIM, 0:DIM] = LAM * DT * W64.T
    wts[DIM, 0:DIM] = LAM * DT * b64
    wts[0:DIM, DIM : 2 * DIM] = (DT / 2) * W64.T
    wts[DIM, DIM : 2 * DIM] = LAM * (DT**2 / 2) * b64
    return np.ascontiguousarray(wts.astype(np.float16))


def _run_device(x0, W, b, **spmd_kwargs):
    if "nc" not in _CACHE:
        _CACHE["nc"] = _build_nc()
    nc = _CACHE["nc"]

    wts = _host_weights(W, b)
    ones = np.ones((1, NT * BL), np.float16)
    in_maps = []
    for i in range(N_CORES):
        shard = np.ascontiguousarray(
            x0[i * BL : (i + 1) * BL].T.astype(np.float16)
        )
        in_maps.append({"x0h": shard, "wth": wts, "oneh": ones})

    return run_bass_kernel_spmd(
        nc, in_maps, core_ids=list(range(N_CORES)), **spmd_kwargs
    )


def kernel(initial_position, W, b):
    x0 = np.asarray(initial_position, np.float32)
    W = np.asarray(W, np.float32)
    b = np.asarray(b, np.float32)

    res = _run_device(x0, W, b)

    out = np.empty((BATCH, T_STEPS, DIM), np.float32)
    for i in range(N_CORES):
        sh = res.results[i]["sh"].astype(np.float32)  # (DIM, NT*BL)
        out[i * BL : (i + 1) * BL, 0] = x0[i * BL : (i + 1) * BL]
        out[i * BL : (i + 1) * BL, 1:] = sh.reshape(DIM, NT, BL).transpose(2, 1, 0)
    return out
